# revision 1
# baseline (speedup 1.0000x reference)
"""Trainium2 Bass kernel for nn_LocalPODLoss (8-core data-parallel).

Algebra: the POD descriptor is linear in the feature map, so
pod(new) - pod(old) = W @ (vec(crop(new)) - vec(crop(old))) for a fixed
matrix W[64, r*r] per scale, where crop is the top-left r x r corner that
the first 32 bilinear output rows/cols can reach (r = 29/15/8 for
h = 56/28/14).  Per scale: ss = sum over images of |W xn - W xo|^2, and
loss = (1e-6 + sum_s sqrt(ss_s)) / 3.

Sharding: batch dim (32) split 4-per-core across 8 cores.  The host packs
each core's cropped images as x[K, 1024] (contraction dim on partitions,
images on the free dim) so the device only does: DMA chunk -> PE matmul
accumulate (+W for new, -W for old) into PSUM -> fused square+reduce on
DVE -> DMA out a [64, 6] partial.  Host sums partials and takes sqrt.
"""

import numpy as np
from contextlib import ExitStack

import concourse.bass as bass
import concourse.tile as tile
from concourse import bacc, mybir
from concourse.bass_utils import run_bass_kernel_spmd

N_CORES = 8
B, C = 32, 256
SIZES = [56, 28, 14]
OUT, HALF = 64, 32
IMGS = (B // N_CORES) * C  # 1024 images per core per scale
F32 = mybir.dt.float32


def _resize_matrix(h):
    import jax, jax.numpy as jnp

    with jax.default_device(jax.devices("cpu")[0]):
        return np.asarray(
            jax.image.resize(jnp.eye(h, dtype=jnp.float32), (OUT, h), method="linear")
        )


def _build_w(h):
    R = _resize_matrix(h).astype(np.float64)
    a = R[:HALF].sum(axis=0) / HALF
    nz = np.nonzero((np.abs(R[:HALF]).sum(axis=0) > 0) | (np.abs(a) > 0))[0]
    r = int(nz.max()) + 1
    Rl, ar = R[:HALF, :r], a[:r]
    W1 = np.einsum("xv,u->xuv", Rl, ar).reshape(HALF, r * r)
    W2 = np.einsum("yu,v->yuv", Rl, ar).reshape(HALF, r * r)
    return np.concatenate([W1, W2], axis=0).astype(np.float32), r


_WS = None  # [(W[64,K], r, K, chunks)] per scale


def _ws():
    global _WS
    if _WS is None:
        ws = []
        for h in SIZES:
            W, r = _build_w(h)
            K = r * r
            chunks = [(st, min(128, K - st)) for st in range(0, K, 128)]
            ws.append((W, r, K, chunks))
        _WS = ws
    return _WS


def _pack_w():
    """One [128, n_blocks*64] f32 array holding every (chunk, +/-) lhsT block."""
    ws = _ws()
    blocks = []
    for W, r, K, chunks in ws:
        Wt = W.T  # [K, 64]
        for st, ck in chunks:
            blocks.append((Wt[st : st + ck], +1.0))
            blocks.append((Wt[st : st + ck], -1.0))
    packed = np.zeros((128, len(blocks) * 64), dtype=np.float32)
    for j, (blk, sign) in enumerate(blocks):
        packed[: blk.shape[0], j * 64 : (j + 1) * 64] = sign * blk
    return packed


_PROG = None  # (nc, input names)


def _build_program():
    ws = _ws()
    nc = bacc.Bacc(
        "TRN2", target_bir_lowering=False, debug=False, num_devices=N_CORES
    )
    x_aps = {}
    for s, (W, r, K, chunks) in enumerate(ws):
        for t in ("n", "o"):
            x_aps[(s, t)] = nc.dram_tensor(
                f"x{s}{t}", [K, IMGS], F32, kind="ExternalInput"
            ).ap()
    n_blocks = sum(len(c) for _, _, _, c in ws) * 2
    wp_ap = nc.dram_tensor("wp", [128, n_blocks * 64], F32, kind="ExternalInput").ap()
    out_ap = nc.dram_tensor("out", [64, 6], F32, kind="ExternalOutput").ap()

    with tile.TileContext(nc) as tc, ExitStack() as ctx:
        wpool = ctx.enter_context(tc.tile_pool(name="w", bufs=1))
        xpool = ctx.enter_context(tc.tile_pool(name="x", bufs=4))
        pspool = ctx.enter_context(tc.tile_pool(name="ps", bufs=6, space="PSUM"))
        spool = ctx.enter_context(tc.tile_pool(name="sq", bufs=2))
        apool = ctx.enter_context(tc.tile_pool(name="acc", bufs=1))

        wbuf = wpool.tile([128, n_blocks * 64], F32)
        nc.sync.dma_start(wbuf[:], wp_ap[:])
        partials = apool.tile([64, 6], F32)

        blk = 0
        for s, (W, r, K, chunks) in enumerate(ws):
            d0 = pspool.tile([64, 512], F32, tag="psd")
            d1 = pspool.tile([64, 512], F32, tag="psd")
            for ci, (st, ck) in enumerate(chunks):
                for ti, t in enumerate(("n", "o")):
                    xt = xpool.tile([ck, IMGS], F32, tag="xt")
                    nc.sync.dma_start(xt[:], x_aps[(s, t)][st : st + ck, :])
                    lhsT = wbuf[0:ck, (blk + ti) * 64 : (blk + ti + 1) * 64]
                    first = ci == 0 and ti == 0
                    last = ci == len(chunks) - 1 and ti == 1
                    nc.tensor.matmul(
                        d0[:], lhsT, xt[:, 0:512], start=first, stop=last
                    )
                    nc.tensor.matmul(
                        d1[:], lhsT, xt[:, 512:1024], start=first, stop=last
                    )
                blk += 2
            for half, d in ((0, d0), (1, d1)):
                dc = spool.tile([64, 512], F32, tag="dc")
                nc.vector.tensor_copy(dc[:], d[:])
                sq = spool.tile([64, 512], F32, tag="sq")
                col = s * 2 + half
                nc.vector.tensor_tensor(
                    out=sq[:], in0=dc[:], in1=dc[:], op=mybir.AluOpType.mult
                )
                nc.vector.tensor_reduce(
                    out=partials[:, col : col + 1],
                    in_=sq[:],
                    axis=mybir.AxisListType.X,
                    op=mybir.AluOpType.add,
                )
        nc.sync.dma_start(out_ap[:], partials[:])

    nc.compile()
    return nc


def _get_program():
    global _PROG
    if _PROG is None:
        _PROG = _build_program()
    return _PROG


_LAST_IN_MAPS = None


def _make_in_maps(inputs):
    ws = _ws()
    wp = _pack_w()
    bpc = B // N_CORES
    in_maps = [{"wp": wp} for _ in range(N_CORES)]
    for s, (W, r, K, chunks) in enumerate(ws):
        for t, key in (("n", f"new_f{s}"), ("o", f"old_f{s}")):
            arr = np.asarray(inputs[key], dtype=np.float32)
            crop = arr[:, :, :r, :r].reshape(B * C, K)
            for i in range(N_CORES):
                sl = crop[i * bpc * C : (i + 1) * bpc * C]  # [1024, K]
                in_maps[i][f"x{s}{t}"] = np.ascontiguousarray(sl.T)
    return in_maps


def _combine(results):
    ss = np.zeros(3, dtype=np.float64)
    for r in results:
        p = r["out"].astype(np.float64)
        for s in range(3):
            ss[s] += p[:, s * 2 : s * 2 + 2].sum()
    loss = (1e-6 + np.sqrt(ss).sum()) / 3.0
    return np.array(loss, dtype=np.float32)


def kernel(**inputs):
    global _LAST_IN_MAPS
    nc = _get_program()
    in_maps = _make_in_maps(inputs)
    _LAST_IN_MAPS = in_maps
    res = run_bass_kernel_spmd(nc, in_maps, list(range(N_CORES)))
    return _combine(res.results)


def profile_last(**kwargs):
    """Re-run the last kernel() invocation with NTFF tracing; returns BassKernelResults."""
    assert _LAST_IN_MAPS is not None, "call kernel() first"
    nc = _get_program()
    return run_bass_kernel_spmd(
        nc, _LAST_IN_MAPS, list(range(N_CORES)), trace=True, **kwargs
    )


def time_device_loop(iters=30):
    """Min per-iteration wall time of the compiled NEFF with device-resident
    inputs (upper bound on HW exec: includes PJRT/axon dispatch)."""
    import time
    import jax
    from concourse import bass2jax as b

    assert _LAST_IN_MAPS is not None, "call kernel() first"
    nc = _get_program()
    b.install_neuronx_cc_hook()

    part_name = nc.partition_id_tensor.name if nc.partition_id_tensor else None
    in_names, out_names, out_avals, zero_outs = [], [], [], []
    for alloc in nc.m.functions[0].allocations:
        if not isinstance(alloc, b.mybir.MemoryLocationSet):
            continue
        name = alloc.memorylocations[0].name
        if alloc.kind == "ExternalInput":
            if name != part_name:
                in_names.append(name)
        elif alloc.kind == "ExternalOutput":
            shape = tuple(alloc.tensor_shape)
            dtype = b.mybir.dt.np(alloc.dtype)
            out_names.append(name)
            out_avals.append(jax.core.ShapedArray(shape, dtype))
            zero_outs.append(np.zeros(shape, dtype))
    n_params = len(in_names)
    all_in_names = in_names + out_names + ([part_name] if part_name else [])

    def _body(*args):
        operands = list(args)
        if part_name is not None:
            operands.append(b.partition_id_tensor())
        return tuple(
            b._bass_exec_p.bind(
                *operands,
                out_avals=tuple(out_avals),
                in_names=tuple(all_in_names),
                out_names=tuple(out_names),
                lowering_input_output_aliases=(),
                sim_require_finite=True,
                sim_require_nnan=True,
                nc=nc,
            )
        )

    devices = jax.devices()[:N_CORES]
    mesh = b.Mesh(np.asarray(devices), ("core",))
    nio = n_params + len(out_names)
    sharded = jax.jit(
        b.shard_map(
            _body,
            mesh=mesh,
            in_specs=(b.PartitionSpec("core"),) * nio,
            out_specs=(b.PartitionSpec("core"),) * len(out_names),
            check_rep=False,
        ),
        keep_unused=True,
    )
    concat_in = [
        np.concatenate([np.asarray(m[nm]) for m in _LAST_IN_MAPS], axis=0)
        for nm in in_names
    ]
    concat_zeros = [
        np.zeros((N_CORES * z.shape[0], *z.shape[1:]), z.dtype) for z in zero_outs
    ]
    sh = jax.sharding.NamedSharding(mesh, b.PartitionSpec("core"))
    dev_in = [jax.device_put(a, sh) for a in concat_in]
    dev_zero = [jax.device_put(a, sh) for a in concat_zeros]
    out = sharded(*dev_in, *dev_zero)  # warm / compile
    jax.block_until_ready(out)
    times = []
    for _ in range(iters):
        t0 = time.perf_counter()
        out = sharded(*dev_in, *dev_zero)
        jax.block_until_ready(out)
        times.append(time.perf_counter() - t0)
    return min(times), sorted(times)[len(times) // 2]

